# revision 7
# baseline (speedup 1.0000x reference)
"""Trainium2 Bass kernel for nn_CustomANFIS (N=4096, D=128, R=256, O=64).

Math (reference):
  memb[n,r,d]  = exp(-(x[n,d]-c[r,d])^2 / (2 s[r,d]^2))
  str[n,r]     = prod_d memb = exp(-q[n,r]) with
                 q[n,r] = sum_d x^2[n,d]*A[d,r] + sum_d x[n,d]*B[d,r] + G[r],
                 A = 1/(2 s^2), B = -c/s^2, G = sum_d c^2/(2 s^2)
  den[n]       = sum_r str + 1e-8
  W[n,r,:]     = x[n,:] @ coeffs[r,:D,:] + coeffs[r,D,:]
  out          = softmax_j( (1/den) * sum_r str[n,r] * W[n,r,j] )

Device algorithm (data-parallel over N across 8 cores), per 128-row n-tile:
  1. strengths^T [r, 128] via 4 accumulating fp32 matmuls (N=128 moving),
     ACT exp (per-partition bias=-G) -> st bf16 [128, 2*128].
  2. den + bias-consequent via 2 bf16 matmuls against cbo, recip on DVE.
  3. T[n, (j,d)] in 8 PSUM chunks of 1024 (4 matmuls each, accumulated
     over the 2 rule k-tiles).  Most chunks: ACT casts PSUM->bf16 SBUF,
     DVE multiplies by x (2x bf16); DIRECT chunks: DVE multiplies
     straight from fp32 PSUM (1x) to offload ACT.
  4. d-reduction: tree levels 1-2 done by accumulate-DMAs (SDMA CCE adds,
     issued on gpsimd), levels 3-7 on DVE in-place in prod.
  5. acc = tree + Tb; softmax over j via ACT exp + accum_out.
"""

import numpy as np
import ml_dtypes

N, D, R, O = 4096, 128, 256, 64
NCORES = 8
NS = N // NCORES          # 512 rows per core
NT = NS // 128            # 4 n-tiles per core
RT = R // 128             # 2 r k-tiles
DJ = D * O                # 8192 (j,d) columns per k-tile
CHUNK = 1024              # T-chunk columns (2 PSUM banks fp32)
NCHUNK = DJ // CHUNK      # 8 chunks (8 j x 128 d each)
JPC = CHUNK // D          # 8 j per chunk

# chunks whose x-multiply reads fp32 PSUM directly on DVE (no ACT cast)
DIRECT_CHUNKS = (3, 7)
# tree levels done by accumulate-DMA (SDMA CCE): 2 -> levels 1 and 2
# NOTE: each accum-DMA must stay <= 32 strided runs per partition (SWDGE
# descriptor limit) -- larger patterns crash the device.
DMA_TREE_LEVELS = 2

_CACHE = {}
BF16 = ml_dtypes.bfloat16


def _build():
    import concourse.bass as bass
    import concourse.tile as tile
    from concourse import bacc, mybir

    f32 = mybir.dt.float32
    f32r = mybir.dt.float32r
    bf16 = mybir.dt.bfloat16
    AF = mybir.ActivationFunctionType
    ALU = mybir.AluOpType
    ts = bass.ts

    nc = bacc.Bacc(
        "TRN2", target_bir_lowering=False, debug=False, num_devices=NCORES
    )

    xt_d = nc.dram_tensor("xt", [D, NS], f32, kind="ExternalInput").ap()
    x2t_d = nc.dram_tensor("x2t", [D, NS], f32, kind="ExternalInput").ap()
    ab_d = nc.dram_tensor("ab", [D, 2 * R], f32, kind="ExternalInput").ap()
    ng_d = nc.dram_tensor("negg", [128, RT], f32, kind="ExternalInput").ap()
    xn_d = nc.dram_tensor("xn", [128, NT * D], bf16, kind="ExternalInput").ap()
    c_d = nc.dram_tensor("cw", [128, NCHUNK * RT * CHUNK], bf16,
                         kind="ExternalInput").ap()
    cbo_d = nc.dram_tensor("cbo", [128, RT * (O + 2)], bf16,
                           kind="ExternalInput").ap()
    out_d = nc.dram_tensor("out", [NS, O], f32, kind="ExternalOutput").ap()

    def r32(ap):
        return ap if ap.dtype == f32r else ap.bitcast(f32r)

    with tile.TileContext(nc) as tc:
        from contextlib import ExitStack

        with ExitStack() as ctx:
            konst = ctx.enter_context(tc.tile_pool(name="konst", bufs=1))
            cw = ctx.enter_context(tc.tile_pool(name="cw", bufs=1))
            stp = ctx.enter_context(tc.tile_pool(name="stp", bufs=2))
            prodp = ctx.enter_context(tc.tile_pool(name="prodp", bufs=2))
            tcpp = ctx.enter_context(tc.tile_pool(name="tcpp", bufs=6))
            small = ctx.enter_context(tc.tile_pool(name="small", bufs=4))
            tpsp = ctx.enter_context(
                tc.tile_pool(name="tpsp", bufs=3, space="PSUM"))
            auxp = ctx.enter_context(
                tc.tile_pool(name="auxp", bufs=2, space="PSUM"))

            # ---- input loads: small first, C chunk-major, 3 queues
            xt_sb = konst.tile([D, NS], f32r)
            x2t_sb = konst.tile([D, NS], f32r)
            ab_sb = konst.tile([D, 2 * R], f32r)
            ng_sb = konst.tile([128, RT], f32)
            xn_sb = konst.tile([128, NT * D], bf16)
            cbo_sb = konst.tile([128, RT * (O + 2)], bf16)
            c_sb = cw.tile([128, NCHUNK * RT * CHUNK], bf16)

            # priority order: tiny/critical tensors head each queue, C after
            nc.sync.dma_start(ng_sb[:], ng_d)
            nc.sync.dma_start(xt_sb[:], xt_d.bitcast(f32r))
            nc.scalar.dma_start(x2t_sb[:], x2t_d.bitcast(f32r))
            nc.gpsimd.dma_start(xn_sb[:], xn_d)
            nc.gpsimd.dma_start(ab_sb[:], ab_d.bitcast(f32r))
            nc.scalar.dma_start(cbo_sb[:], cbo_d)
            c_q = [nc.sync, nc.scalar, nc.gpsimd]
            for c in range(NCHUNK):
                eng = c_q[c % 3]
                sl = slice(c * RT * CHUNK, (c + 1) * RT * CHUNK)
                eng.dma_start(c_sb[:, sl], c_d[:, sl])

            # warm the exp table during the DMA head
            dum = small.tile([128, 1], f32, name="dum")
            nc.vector.memset(dum[:], 0.0)
            dume = small.tile([128, 1], f32, name="dume")
            nc.scalar.activation(dume[:], dum[:], AF.Exp)

            xrows = xn_sb[:].rearrange("p (t d) -> p t d", t=NT)

            for nt in range(NT):
                nsl = slice(nt * 128, (nt + 1) * 128)

                # ---- strengths: q = B@x + A@x2 per rule k-tile
                aux = auxp.tile([128, 512], f32, name=f"aux{nt}", tag="aux")
                for rt in range(RT):
                    bsl = slice(R + rt * 128, R + (rt + 1) * 128)
                    asl = slice(rt * 128, (rt + 1) * 128)
                    osl = slice(rt * 128, (rt + 1) * 128)
                    nc.tensor.matmul(
                        aux[:, osl], ab_sb[:, bsl], xt_sb[:, nsl],
                        start=True, stop=False,
                    )
                    nc.tensor.matmul(
                        aux[:, osl], ab_sb[:, asl], x2t_sb[:, nsl],
                        start=False, stop=True,
                    )
                st = stp.tile([128, R], bf16, name=f"st{nt}", tag="st")
                for rt in range(RT):
                    nc.scalar.activation(
                        st[:, ts(rt, 128)], aux[:, ts(rt, 128)], AF.Exp,
                        bias=ng_sb[:, rt : rt + 1], scale=-1.0,
                    )

                # ---- den + bias consequent
                for rt in range(RT):
                    nc.tensor.matmul(
                        aux[:, 256 : 256 + O + 2], st[:, ts(rt, 128)],
                        cbo_sb[:, ts(rt, O + 2)],
                        start=(rt == 0), stop=(rt == RT - 1),
                    )
                denc = small.tile([128, 1], f32, name=f"denc{nt}")
                nc.vector.tensor_scalar_add(denc[:], aux[:, 256:257], 1e-8)
                scalec = small.tile([128, 1], f32, name=f"scalec{nt}")
                nc.vector.reciprocal(scalec[:], denc[:])
                tb_sb = small.tile([128, O], f32, name=f"tb{nt}", tag="tb")
                nc.scalar.activation(tb_sb[:], aux[:, 258 : 258 + O], AF.Copy)

                # ---- T chunks + x-multiply
                prod = prodp.tile([128, O, D], bf16, name=f"prod{nt}", tag="prod")
                xrow = xrows[:, nt, :]
                xb = xrow.unsqueeze(1).broadcast_to([128, JPC, D])

                for c in range(NCHUNK):
                    tps = tpsp.tile([128, CHUNK], f32, name=f"tps{nt}_{c}", tag="tps")
                    for rt in range(RT):
                        for half in range(2):
                            csl = slice(
                                c * RT * CHUNK + rt * CHUNK + half * 512,
                                c * RT * CHUNK + rt * CHUNK + half * 512 + 512,
                            )
                            nc.tensor.matmul(
                                tps[:, half * 512 : (half + 1) * 512],
                                st[:, ts(rt, 128)], c_sb[:, csl],
                                start=(rt == 0), stop=(rt == RT - 1),
                            )
                    tview = tps[:].rearrange("p (j d) -> p j d", j=JPC)
                    oview = prod[:, c * JPC : (c + 1) * JPC, :]
                    if c in DIRECT_CHUNKS:
                        nc.vector.tensor_tensor(oview, tview, xb, ALU.mult)
                    else:
                        tcp = tcpp.tile([128, JPC, D], bf16, name=f"tcp{nt}_{c}", tag="tcp")
                        nc.scalar.activation(tcp[:], tps[:], AF.Copy)
                        nc.vector.tensor_tensor(oview, tcp[:], xb, ALU.mult)

                    # tree level 1 by halves as soon as inputs are ready
                    if DMA_TREE_LEVELS >= 1 and c == NCHUNK // 2 - 1:
                        nc.gpsimd.dma_start(
                            prod[:, 0 : O // 2, 0 : D // 2],
                            prod[:, 0 : O // 2, D // 2 : D],
                            accum_op=ALU.add,
                        )
                if DMA_TREE_LEVELS >= 1:
                    nc.gpsimd.dma_start(
                        prod[:, O // 2 : O, 0 : D // 2],
                        prod[:, O // 2 : O, D // 2 : D],
                        accum_op=ALU.add,
                    )
                    h0 = D // 2
                else:
                    h0 = D
                if DMA_TREE_LEVELS >= 2:
                    for jh in range(2):
                        nc.gpsimd.dma_start(
                            prod[:, jh * 32 : (jh + 1) * 32, 0 : D // 4],
                            prod[:, jh * 32 : (jh + 1) * 32, D // 4 : D // 2],
                            accum_op=ALU.add,
                        )
                    h0 = D // 4

                # remaining tree levels in-place on DVE
                h = h0
                while h > 1:
                    h //= 2
                    nc.vector.tensor_tensor(
                        prod[:, :, 0:h], prod[:, :, 0:h], prod[:, :, h : 2 * h],
                        ALU.add,
                    )

                # acc = tree + Tb
                acc = small.tile([128, O], f32, name=f"acc{nt}")
                nc.vector.scalar_tensor_tensor(
                    acc[:], prod[:, :, 0], 1.0, tb_sb[:], ALU.mult, ALU.add
                )

                # softmax over j of logits = acc/den (fused scale into exp)
                negm = small.tile([128, 1], f32, name=f"negm{nt}")
                nc.vector.tensor_reduce(
                    negm[:], acc[:], axis=mybir.AxisListType.X, op=ALU.max,
                    negate=True,
                )
                negmb = small.tile([128, 1], f32, name=f"negmb{nt}")
                nc.vector.tensor_tensor(negmb[:], negm[:], scalec[:], ALU.mult)
                exps = small.tile([128, O], f32, name=f"exps{nt}")
                sume = small.tile([128, 1], f32, name=f"sume{nt}")
                nc.scalar.activation(
                    exps[:], acc[:], AF.Exp, bias=negmb[:], scale=scalec[:],
                    accum_out=sume[:],
                )
                rs = small.tile([128, 1], f32, name=f"rs{nt}")
                nc.vector.reciprocal(rs[:], sume[:])
                osb = small.tile([128, O], f32, name=f"osb{nt}")
                nc.scalar.activation(osb[:], exps[:], AF.Copy, scale=rs[:])
                nc.sync.dma_start(out_d[nsl, :], osb[:])

    nc.compile()
    return nc


def _prep_inputs(X, centers, sigmas, coeffs):
    """Host-side sharding + layout transforms (numpy only)."""
    X = np.ascontiguousarray(X, dtype=np.float32)
    centers = np.asarray(centers, dtype=np.float32)
    sigmas = np.asarray(sigmas, dtype=np.float32)
    coeffs = np.asarray(coeffs, dtype=np.float32)

    inv2s2 = 1.0 / (2.0 * sigmas * sigmas)            # [R, D]
    A = inv2s2.T                                       # [D, R]
    B = (-centers / (sigmas * sigmas)).T               # [D, R]
    AB = np.ascontiguousarray(np.concatenate([A, B], axis=1))  # [D, 2R]
    G = (centers * centers * inv2s2).sum(axis=1)       # [R]
    negG = np.ascontiguousarray(-G.reshape(RT, 128).T)  # [128, RT]

    # C in [128 r-part, chunk, rt, cols] layout, bf16
    Cjd = np.ascontiguousarray(coeffs[:, :D, :].transpose(0, 2, 1))  # [R, O, D]
    Ck = Cjd.reshape(RT, 128, DJ)                     # [rt, r, (j d)]
    Cdev = np.ascontiguousarray(
        Ck.reshape(RT, 128, NCHUNK, CHUNK).transpose(1, 2, 0, 3)
        .reshape(128, NCHUNK * RT * CHUNK)
    ).astype(BF16)
    Cb = coeffs[:, D, :].reshape(RT, 128, O).transpose(1, 0, 2)  # [128, RT, O]
    Cbo = np.ones((128, RT, O + 2), dtype=np.float32)
    Cbo[:, :, 2:] = Cb
    Cbo = np.ascontiguousarray(Cbo.reshape(128, RT * (O + 2))).astype(BF16)

    in_maps = []
    for i in range(NCORES):
        Xs = X[i * NS : (i + 1) * NS]                  # [512, 128]
        xt = np.ascontiguousarray(Xs.T)                # [128, 512]
        x2t = np.ascontiguousarray(xt * xt)
        xn = np.ascontiguousarray(
            Xs.reshape(NT, 128, D).transpose(1, 0, 2).reshape(128, NT * D)
        ).astype(BF16)
        in_maps.append(
            {
                "xt": xt,
                "x2t": x2t,
                "ab": AB,
                "negg": negG,
                "xn": xn,
                "cw": Cdev,
                "cbo": Cbo,
            }
        )
    return in_maps


def kernel(X, centers, sigmas, coeffs):
    from concourse.bass_utils import run_bass_kernel_spmd

    if "nc" not in _CACHE:
        _CACHE["nc"] = _build()
    nc = _CACHE["nc"]

    in_maps = _prep_inputs(X, centers, sigmas, coeffs)
    res = run_bass_kernel_spmd(nc, in_maps, list(range(NCORES)))
    out = np.concatenate([res.results[i]["out"] for i in range(NCORES)], axis=0)
    return out.astype(np.float32)


if __name__ == "__main__":
    rng = np.random.default_rng(0)
    X = rng.standard_normal((N, D), dtype=np.float32)
    centers = 0.5 * rng.standard_normal((R, D)).astype(np.float32)
    sigmas = (1.5 + rng.random((R, D))).astype(np.float32)
    coeffs = (0.02 * rng.standard_normal((R, D + 1, O))).astype(np.float32)
    out = kernel(X=X, centers=centers, sigmas=sigmas, coeffs=coeffs)
    print(out.shape, out.dtype, out.sum(axis=1)[:4])


# revision 9
# speedup vs baseline: 2.7435x; 2.7435x over previous
"""Trainium2 Bass kernel for nn_CustomANFIS (N=4096, D=128, R=256, O=64).

Math (reference):
  memb[n,r,d]  = exp(-(x[n,d]-c[r,d])^2 / (2 s[r,d]^2))
  str[n,r]     = prod_d memb = exp(-q[n,r]) with
                 q[n,r] = sum_d x^2[n,d]*A[d,r] + sum_d x[n,d]*B[d,r] + G[r],
                 A = 1/(2 s^2), B = -c/s^2, G = sum_d c^2/(2 s^2)
  den[n]       = sum_r str + 1e-8
  W[n,r,:]     = x[n,:] @ coeffs[r,:D,:] + coeffs[r,D,:]
  out          = softmax_j( (1/den) * sum_r str[n,r] * W[n,r,j] )

Device algorithm (data-parallel over N across 8 cores), per 128-row n-tile:
  1. strengths^T [r, 128] via 4 accumulating fp32 matmuls (N=128 moving),
     ACT exp (per-partition bias=-G) -> st bf16 [128, 2*128].
  2. den + bias-consequent via 2 bf16 matmuls against cbo, recip on DVE.
  3. T[n, (j,d)] in 8 PSUM chunks of 1024 (4 matmuls each, accumulated
     over the 2 rule k-tiles).  Most chunks: ACT casts PSUM->bf16 SBUF,
     DVE multiplies by x (2x bf16); DIRECT chunks: DVE multiplies
     straight from fp32 PSUM (1x) to offload ACT.
  4. d-reduction: tree levels 1-2 done by accumulate-DMAs (SDMA CCE adds,
     issued on gpsimd), levels 3-7 on DVE in-place in prod.
  5. acc = tree + Tb; softmax over j via ACT exp + accum_out.
"""

import numpy as np
import ml_dtypes

N, D, R, O = 4096, 128, 256, 64
NCORES = 8
NS = N // NCORES          # 512 rows per core
NT = NS // 128            # 4 n-tiles per core
RT = R // 128             # 2 r k-tiles
DJ = D * O                # 8192 (j,d) columns per k-tile
CHUNK = 1024              # T-chunk columns (2 PSUM banks fp32)
NCHUNK = DJ // CHUNK      # 8 chunks (8 j x 128 d each)
JPC = CHUNK // D          # 8 j per chunk

# chunks whose x-multiply reads fp32 PSUM directly on DVE (no ACT cast)
DIRECT_CHUNKS = (3, 7)
# tree levels done by accumulate-DMA (SDMA CCE): 2 -> levels 1 and 2
# NOTE: each accum-DMA must stay <= 32 strided runs per partition (SWDGE
# descriptor limit) -- larger patterns crash the device.
DMA_TREE_LEVELS = 2

_CACHE = {}
BF16 = ml_dtypes.bfloat16


def _build():
    import concourse.bass as bass
    import concourse.tile as tile
    from concourse import bacc, mybir

    f32 = mybir.dt.float32
    f32r = mybir.dt.float32r
    bf16 = mybir.dt.bfloat16
    AF = mybir.ActivationFunctionType
    ALU = mybir.AluOpType
    ts = bass.ts

    nc = bacc.Bacc(
        "TRN2", target_bir_lowering=False, debug=False, num_devices=NCORES
    )

    xt_d = nc.dram_tensor("xt", [D, NS], f32, kind="ExternalInput").ap()
    x2t_d = nc.dram_tensor("x2t", [D, NS], f32, kind="ExternalInput").ap()
    ab_d = nc.dram_tensor("ab", [D, 2 * R], f32, kind="ExternalInput").ap()
    ng_d = nc.dram_tensor("negg", [128, RT], f32, kind="ExternalInput").ap()
    xn_d = nc.dram_tensor("xn", [128, NT * D], bf16, kind="ExternalInput").ap()
    c_d = nc.dram_tensor("cw", [128, NCHUNK * RT * CHUNK], bf16,
                         kind="ExternalInput").ap()
    cbo_d = nc.dram_tensor("cbo", [128, RT * (O + 2)], bf16,
                           kind="ExternalInput").ap()
    out_d = nc.dram_tensor("out", [NS, O], f32, kind="ExternalOutput").ap()

    def r32(ap):
        return ap if ap.dtype == f32r else ap.bitcast(f32r)

    with tile.TileContext(nc) as tc:
        from contextlib import ExitStack

        with ExitStack() as ctx:
            konst = ctx.enter_context(tc.tile_pool(name="konst", bufs=1))
            cw = ctx.enter_context(tc.tile_pool(name="cw", bufs=1))
            stp = ctx.enter_context(tc.tile_pool(name="stp", bufs=2))
            prodp = ctx.enter_context(tc.tile_pool(name="prodp", bufs=2))
            tcpp = ctx.enter_context(tc.tile_pool(name="tcpp", bufs=6))
            small = ctx.enter_context(tc.tile_pool(name="small", bufs=4))
            tpsp = ctx.enter_context(
                tc.tile_pool(name="tpsp", bufs=3, space="PSUM"))
            auxp = ctx.enter_context(
                tc.tile_pool(name="auxp", bufs=2, space="PSUM"))

            # ---- input loads: small first, C chunk-major, 3 queues
            xt_sb = konst.tile([D, NS], f32r)
            x2t_sb = konst.tile([D, NS], f32r)
            ab_sb = konst.tile([D, 2 * R], f32r)
            ng_sb = konst.tile([128, RT], f32)
            xn_sb = konst.tile([128, NT * D], bf16)
            cbo_sb = konst.tile([128, RT * (O + 2)], bf16)
            c_sb = cw.tile([128, NCHUNK * RT * CHUNK], bf16)

            # priority order: tiny/critical tensors head each queue, C after
            nc.sync.dma_start(ng_sb[:], ng_d)
            nc.sync.dma_start(xt_sb[:], xt_d.bitcast(f32r))
            nc.scalar.dma_start(x2t_sb[:], x2t_d.bitcast(f32r))
            nc.gpsimd.dma_start(xn_sb[:], xn_d)
            nc.gpsimd.dma_start(ab_sb[:], ab_d.bitcast(f32r))
            nc.scalar.dma_start(cbo_sb[:], cbo_d)
            c_q = [nc.sync, nc.scalar, nc.gpsimd]
            for c in range(NCHUNK):
                eng = c_q[c % 3]
                sl = slice(c * RT * CHUNK, (c + 1) * RT * CHUNK)
                eng.dma_start(c_sb[:, sl], c_d[:, sl])

            # warm the exp table during the DMA head
            dum = small.tile([128, 1], f32, name="dum")
            nc.vector.memset(dum[:], 0.0)
            dume = small.tile([128, 1], f32, name="dume")
            nc.scalar.activation(dume[:], dum[:], AF.Exp)

            xrows = xn_sb[:].rearrange("p (t d) -> p t d", t=NT)

            for nt in range(NT):
                nsl = slice(nt * 128, (nt + 1) * 128)

                # ---- strengths: q = B@x + A@x2 per rule k-tile
                aux = auxp.tile([128, 512], f32, name=f"aux{nt}", tag="aux")
                for rt in range(RT):
                    bsl = slice(R + rt * 128, R + (rt + 1) * 128)
                    asl = slice(rt * 128, (rt + 1) * 128)
                    osl = slice(rt * 128, (rt + 1) * 128)
                    nc.tensor.matmul(
                        aux[:, osl], ab_sb[:, bsl], xt_sb[:, nsl],
                        start=True, stop=False,
                    )
                    nc.tensor.matmul(
                        aux[:, osl], ab_sb[:, asl], x2t_sb[:, nsl],
                        start=False, stop=True,
                    )
                st = stp.tile([128, R], bf16, name=f"st{nt}", tag="st")
                for rt in range(RT):
                    nc.scalar.activation(
                        st[:, ts(rt, 128)], aux[:, ts(rt, 128)], AF.Exp,
                        bias=ng_sb[:, rt : rt + 1], scale=-1.0,
                    )

                # ---- den + bias consequent
                for rt in range(RT):
                    nc.tensor.matmul(
                        aux[:, 256 : 256 + O + 2], st[:, ts(rt, 128)],
                        cbo_sb[:, ts(rt, O + 2)],
                        start=(rt == 0), stop=(rt == RT - 1),
                    )
                denc = small.tile([128, 1], f32, name=f"denc{nt}")
                nc.vector.tensor_scalar_add(denc[:], aux[:, 256:257], 1e-8)
                scalec = small.tile([128, 1], f32, name=f"scalec{nt}")
                nc.vector.reciprocal(scalec[:], denc[:])
                tb_sb = small.tile([128, O], f32, name=f"tb{nt}", tag="tb")
                nc.scalar.activation(tb_sb[:], aux[:, 258 : 258 + O], AF.Copy)

                # ---- T chunks + x-multiply
                # prod layout: [p, (dhm 4, j 64, dq 32)] -- d = dhm*32+dq.
                # Tree levels 1 and 2 are then FLAT column halves (single-run
                # DMA/DVE patterns): lvl1 cols 0:4096 += 4096:8192, lvl2
                # cols 0:2048 += 2048:4096.
                prod = prodp.tile([128, O * D], bf16, name=f"prod{nt}", tag="prod")
                pj = prod[:].rearrange("p (m j q) -> p j m q", m=4, j=O)
                xrow = xrows[:, nt, :]
                xb = (
                    xrow.rearrange("p (m q) -> p m q", m=4)
                    .unsqueeze(1)
                    .broadcast_to([128, JPC, 4, 32])
                )

                for c in range(NCHUNK):
                    tps = tpsp.tile([128, CHUNK], f32, name=f"tps{nt}_{c}", tag="tps")
                    for rt in range(RT):
                        for half in range(2):
                            csl = slice(
                                c * RT * CHUNK + rt * CHUNK + half * 512,
                                c * RT * CHUNK + rt * CHUNK + half * 512 + 512,
                            )
                            nc.tensor.matmul(
                                tps[:, half * 512 : (half + 1) * 512],
                                st[:, ts(rt, 128)], c_sb[:, csl],
                                start=(rt == 0), stop=(rt == RT - 1),
                            )
                    tview = tps[:].rearrange("p (j m q) -> p j m q", j=JPC, m=4)
                    oview = pj[:, c * JPC : (c + 1) * JPC, :, :]
                    if c in DIRECT_CHUNKS:
                        nc.vector.tensor_tensor(oview, tview, xb, ALU.mult)
                    else:
                        tcp = tcpp.tile([128, JPC, D], bf16, name=f"tcp{nt}_{c}", tag="tcp")
                        nc.scalar.activation(tcp[:], tps[:], AF.Copy)
                        tcv = tcp[:].rearrange("p j (m q) -> p j m q", m=4)
                        nc.vector.tensor_tensor(oview, tcv, xb, ALU.mult)

                # tree levels 1-2: flat contiguous halves
                # (CCE max element count is 2048 -> split level 1 in two)
                HD = O * D // 2
                if DMA_TREE_LEVELS >= 1:
                    for hh in range(2):
                        nc.gpsimd.dma_start(
                            prod[:, hh * 2048 : (hh + 1) * 2048],
                            prod[:, HD + hh * 2048 : HD + (hh + 1) * 2048],
                            accum_op=ALU.add,
                        )
                else:
                    nc.vector.tensor_tensor(
                        prod[:, 0:HD], prod[:, 0:HD], prod[:, HD : 2 * HD],
                        ALU.add,
                    )
                if DMA_TREE_LEVELS >= 2:
                    nc.gpsimd.dma_start(
                        prod[:, 0 : HD // 2], prod[:, HD // 2 : HD],
                        accum_op=ALU.add,
                    )
                else:
                    nc.vector.tensor_tensor(
                        prod[:, 0 : HD // 2], prod[:, 0 : HD // 2],
                        prod[:, HD // 2 : HD], ALU.add,
                    )

                # remaining tree levels in-place on DVE: [p, j 64, q 32]
                t3 = prod[:, 0 : HD // 2].rearrange("p (j q) -> p j q", j=O)
                h = 32
                while h > 1:
                    h //= 2
                    nc.vector.tensor_tensor(
                        t3[:, :, 0:h], t3[:, :, 0:h], t3[:, :, h : 2 * h],
                        ALU.add,
                    )

                # acc = tree + Tb
                acc = small.tile([128, O], f32, name=f"acc{nt}")
                nc.vector.scalar_tensor_tensor(
                    acc[:], t3[:, :, 0], 1.0, tb_sb[:], ALU.mult, ALU.add
                )

                # softmax over j of logits = acc/den (fused scale into exp)
                negm = small.tile([128, 1], f32, name=f"negm{nt}")
                nc.vector.tensor_reduce(
                    negm[:], acc[:], axis=mybir.AxisListType.X, op=ALU.max,
                    negate=True,
                )
                negmb = small.tile([128, 1], f32, name=f"negmb{nt}")
                nc.vector.tensor_tensor(negmb[:], negm[:], scalec[:], ALU.mult)
                exps = small.tile([128, O], f32, name=f"exps{nt}")
                sume = small.tile([128, 1], f32, name=f"sume{nt}")
                nc.scalar.activation(
                    exps[:], acc[:], AF.Exp, bias=negmb[:], scale=scalec[:],
                    accum_out=sume[:],
                )
                rs = small.tile([128, 1], f32, name=f"rs{nt}")
                nc.vector.reciprocal(rs[:], sume[:])
                osb = small.tile([128, O], f32, name=f"osb{nt}")
                nc.scalar.activation(osb[:], exps[:], AF.Copy, scale=rs[:])
                nc.sync.dma_start(out_d[nsl, :], osb[:])

    nc.compile()
    return nc


def _prep_inputs(X, centers, sigmas, coeffs):
    """Host-side sharding + layout transforms (numpy only)."""
    X = np.ascontiguousarray(X, dtype=np.float32)
    centers = np.asarray(centers, dtype=np.float32)
    sigmas = np.asarray(sigmas, dtype=np.float32)
    coeffs = np.asarray(coeffs, dtype=np.float32)

    inv2s2 = 1.0 / (2.0 * sigmas * sigmas)            # [R, D]
    A = inv2s2.T                                       # [D, R]
    B = (-centers / (sigmas * sigmas)).T               # [D, R]
    AB = np.ascontiguousarray(np.concatenate([A, B], axis=1))  # [D, 2R]
    G = (centers * centers * inv2s2).sum(axis=1)       # [R]
    negG = np.ascontiguousarray(-G.reshape(RT, 128).T)  # [128, RT]

    # C in [128 r-part, chunk, rt, cols] layout, bf16
    Cjd = np.ascontiguousarray(coeffs[:, :D, :].transpose(0, 2, 1))  # [R, O, D]
    Ck = Cjd.reshape(RT, 128, DJ)                     # [rt, r, (j d)]
    Cdev = np.ascontiguousarray(
        Ck.reshape(RT, 128, NCHUNK, CHUNK).transpose(1, 2, 0, 3)
        .reshape(128, NCHUNK * RT * CHUNK)
    ).astype(BF16)
    Cb = coeffs[:, D, :].reshape(RT, 128, O).transpose(1, 0, 2)  # [128, RT, O]
    Cbo = np.ones((128, RT, O + 2), dtype=np.float32)
    Cbo[:, :, 2:] = Cb
    Cbo = np.ascontiguousarray(Cbo.reshape(128, RT * (O + 2))).astype(BF16)

    in_maps = []
    for i in range(NCORES):
        Xs = X[i * NS : (i + 1) * NS]                  # [512, 128]
        xt = np.ascontiguousarray(Xs.T)                # [128, 512]
        x2t = np.ascontiguousarray(xt * xt)
        xn = np.ascontiguousarray(
            Xs.reshape(NT, 128, D).transpose(1, 0, 2).reshape(128, NT * D)
        ).astype(BF16)
        in_maps.append(
            {
                "xt": xt,
                "x2t": x2t,
                "ab": AB,
                "negg": negG,
                "xn": xn,
                "cw": Cdev,
                "cbo": Cbo,
            }
        )
    return in_maps


def kernel(X, centers, sigmas, coeffs):
    from concourse.bass_utils import run_bass_kernel_spmd

    if "nc" not in _CACHE:
        _CACHE["nc"] = _build()
    nc = _CACHE["nc"]

    in_maps = _prep_inputs(X, centers, sigmas, coeffs)
    res = run_bass_kernel_spmd(nc, in_maps, list(range(NCORES)))
    out = np.concatenate([res.results[i]["out"] for i in range(NCORES)], axis=0)
    return out.astype(np.float32)


if __name__ == "__main__":
    rng = np.random.default_rng(0)
    X = rng.standard_normal((N, D), dtype=np.float32)
    centers = 0.5 * rng.standard_normal((R, D)).astype(np.float32)
    sigmas = (1.5 + rng.random((R, D))).astype(np.float32)
    coeffs = (0.02 * rng.standard_normal((R, D + 1, O))).astype(np.float32)
    out = kernel(X=X, centers=centers, sigmas=sigmas, coeffs=coeffs)
    print(out.shape, out.dtype, out.sum(axis=1)[:4])
